# revision 10
# baseline (speedup 1.0000x reference)
"""Trainium2 Bass kernel for a single causal attention head.

Problem: x [8, 2048, 1024] f32, Wq/Wk/Wv [1024, 64] f32 ->
         out [8, 2048, 64] f32  (causal softmax(q k^T / 8) v per batch)

Strategy: data-parallel over batch -- core b computes batch element b,
no collectives. Per core, a column-pipelined flash-style kernel:
the T axis is split in 4 slices of 512; loading x^T slice j unlocks
the projections, score column j and PV windows of column j, so DMA
overlaps compute across columns.

Layouts (bf16 compute, f32 PSUM accumulation):
  x^T      [E=1024, T] in DRAM (host pre-transposed), cast to bf16 on load
  qT | kT  [64, T] each, computed as one M=128 matmul with lhsT=[Wq|Wk]
  scoresT  [s-chunk 128, t 512] = kT_slice^T-free matmul (K=h=64)
  exp      bf16 tiles, causal staircase handled by block skipping +
           one [128,128] lower-triangle multiplicative mask
  PV       out[t 128, 65] = expT_block^T @ [v | 1]; column 64 gives the
           softmax denominator for free; normalize via reciprocal +
           per-partition tensor_scalar multiply.
"""

import sys
from contextlib import ExitStack

sys.path.insert(0, "/opt/trn_rl_repo")

import numpy as np

import concourse.bass as bass
import concourse.tile as tile
from concourse import bacc, mybir
from concourse.bass_utils import run_bass_kernel_spmd

B, T, E, H = 8, 2048, 1024, 64
NCORES = 8
TJ = 512            # t-slice width (score tile free dim)
NJ = T // TJ        # 4 columns
SC = T // 128       # 16 s-chunks
BF16 = mybir.dt.bfloat16
F32 = mybir.dt.float32


def build_kernel(tc: "tile.TileContext", out: bass.AP, xT: bass.AP, wqkv: bass.AP):
    nc = tc.nc
    EXP = mybir.ActivationFunctionType.Exp
    MUL = mybir.AluOpType.mult

    ctx = ExitStack()
    const = ctx.enter_context(tc.tile_pool(name="const", bufs=1))
    xp = ctx.enter_context(tc.tile_pool(name="xp", bufs=2))
    expp = ctx.enter_context(tc.tile_pool(name="expp", bufs=20))
    outp = ctx.enter_context(tc.tile_pool(name="outp", bufs=2))
    small = ctx.enter_context(tc.tile_pool(name="small", bufs=4))
    ps_qk = ctx.enter_context(tc.tile_pool(name="ps_qk", bufs=2, space="PSUM"))
    ps_v = ctx.enter_context(tc.tile_pool(name="ps_v", bufs=2, space="PSUM"))
    ps_s = ctx.enter_context(tc.tile_pool(name="ps_s", bufs=2, space="PSUM"))
    ps_o = ctx.enter_context(tc.tile_pool(name="ps_o", bufs=2, space="PSUM"))

    # Weights: [128, ec, r, h] so [Wq|Wk] for one e-chunk is a contiguous
    # 128-wide free slice (lhsT of the combined qk projection).
    w_sb = const.tile([128, 8, 3, H], BF16, tag="w")
    for r in range(3):
        nc.gpsimd.dma_start(
            w_sb[:, :, r, :], wqkv[r].rearrange("(ec p) h -> p ec h", p=128)
        )

    # Lower-triangular multiplicative mask: tri[p, f] = 1 if p <= f else 0.
    tri = const.tile([128, 128], BF16, tag="tri")
    nc.gpsimd.memset(tri[:], 0.0)
    # keep 0 where p > f, fill 1.0 where p <= f  (fill=0.0 would hit the
    # uninitialized Pool_zero register, so build the mask inverted)
    nc.gpsimd.affine_select(
        out=tri[:], in_=tri[:],
        compare_op=mybir.AluOpType.is_ge, fill=1.0,
        base=-1, pattern=[[-1, 128]], channel_multiplier=1,
    )

    qk_tiles = []   # per column: [128, TJ] bf16, rows 0:64 = qT, 64:128 = kT
    kt_tiles = []   # per column: [64, TJ] bf16 (kT shifted to partitions 0-63)
    v_tiles = []    # per column: [128, 4, H+1] bf16 ([v | ones])
    for j in range(NJ):
        qk_tiles.append(const.tile([128, TJ], BF16, tag=f"qk{j}", name=f"qk{j}"))
        kt_tiles.append(const.tile([64, TJ], BF16, tag=f"kt{j}", name=f"kt{j}"))
        v_tiles.append(const.tile([128, 4, H + 1], BF16, tag=f"v{j}", name=f"v{j}"))

    for j in range(NJ):
        jsl = slice(j * TJ, (j + 1) * TJ)
        qk_j, kt_j, v_j = qk_tiles[j], kt_tiles[j], v_tiles[j]

        # --- load x^T slice j (all 8 e-chunks of t-range jsl), cast bf16
        xsl = xp.tile([128, 8, TJ], BF16, tag="x")
        nc.gpsimd.dma_start(xsl[:], xT[:, jsl].rearrange("(ec p) t -> p ec t", p=128))

        # --- combined q/k projection: psum[0:64]=qT, [64:128]=kT
        psq = ps_qk.tile([128, TJ], F32, tag="qk")
        for ec in range(8):
            nc.tensor.matmul(
                psq[:], w_sb[:, ec, 0:2, :], xsl[:, ec, :],
                start=(ec == 0), stop=(ec == 7),
            )
        nc.vector.tensor_copy(qk_j[:], psq[:])
        # move kT rows down to partitions 0-63 (matmul lhsT needs base 0)
        nc.sync.dma_start(kt_j[:], qk_j[64:128, :])

        # --- v projection for s-chunks 4j..4j+3, plus ones column
        nc.vector.memset(v_j[:, :, H], 1.0)
        for c in range(4):
            psv = ps_v.tile([128, H], F32, tag="v")
            for ec in range(8):
                nc.tensor.matmul(
                    psv[:], xsl[:, ec, c * 128:(c + 1) * 128], w_sb[:, ec, 2, :],
                    start=(ec == 0), stop=(ec == 7),
                )
            nc.vector.tensor_copy(v_j[:, c, 0:H], psv[:])

        # --- score column j: scoresT[s-chunk i, t in jsl], exp, mask
        etiles = []
        for i in range(4 * j + 4):
            r = i - 4 * j          # r >= 0 -> staircase block
            f0 = 128 * r if r > 0 else 0
            pss = ps_s.tile([128, TJ], F32, tag="s")
            nc.tensor.matmul(
                pss[:, f0:], kt_tiles[i // 4][:, (i % 4) * 128:(i % 4 + 1) * 128],
                qk_j[0:64, f0:], start=True, stop=True,
            )
            e = expp.tile([128, TJ], BF16, tag="e", name=f"e{j}_{i}")
            nc.scalar.activation(e[:, f0:], pss[:, f0:], EXP, scale=0.125)
            if r >= 0:
                # only the diagonal 128-wide window is partially valid
                nc.vector.tensor_tensor(
                    e[:, 128 * r:128 * (r + 1)], e[:, 128 * r:128 * (r + 1)],
                    tri[:], op=MUL,
                )
            etiles.append(e)

        # --- PV for the 4 output windows of column j
        osb = outp.tile([128, 4, H], F32, tag="o")
        for c in range(4):
            m = 4 * j + c
            po = ps_o.tile([128, H + 1], F32, tag="po")
            for i in range(m + 1):
                nc.tensor.matmul(
                    po[:], etiles[i][:, c * 128:(c + 1) * 128],
                    v_tiles[i // 4][:, i % 4, :],
                    start=(i == 0), stop=(i == m),
                )
            rec = small.tile([128, 1], F32, tag="rec")
            nc.vector.reciprocal(rec[:], po[:, H:H + 1])
            nc.vector.tensor_scalar_mul(osb[:, c, :], po[:, 0:H], rec[:])
        nc.sync.dma_start(out[jsl, :].rearrange("(c p) h -> p c h", p=128), osb[:])

    ctx.close()


_NC_CACHE = None


def build_nc():
    global _NC_CACHE
    if _NC_CACHE is not None:
        return _NC_CACHE
    nc = bacc.Bacc(
        "TRN2", target_bir_lowering=False, debug=False,
        enable_asserts=False, num_devices=NCORES,
    )
    xT = nc.dram_tensor("xT", [E, T], F32, kind="ExternalInput").ap()
    wqkv = nc.dram_tensor("wqkv", [3, E, H], F32, kind="ExternalInput").ap()
    out = nc.dram_tensor("out", [T, H], F32, kind="ExternalOutput").ap()
    with tile.TileContext(nc) as tc:
        build_kernel(tc, out, xT, wqkv)
    nc.finalize()
    _NC_CACHE = nc
    return nc


def _install_profile_hook():
    """The agent image lacks ``antenv.axon_hooks``; inject a shim so
    run_bass_kernel_spmd(trace=True) can reach the axon NTFF profiler."""
    import types

    if "antenv.axon_hooks" not in sys.modules:
        mod = types.ModuleType("antenv.axon_hooks")
        holder = {}
        mod.set_axon_ntff_profile_hook = lambda h: holder.__setitem__("h", h)
        mod.get_axon_ntff_profile_hook = lambda: holder.get("h")
        sys.modules["antenv.axon_hooks"] = mod
    from trn_agent_boot.trn_boot import _ntff_profile_via_ctypes

    hook = _ntff_profile_via_ctypes("/opt/axon/libaxon_pjrt.so")
    sys.modules["antenv.axon_hooks"].set_axon_ntff_profile_hook(hook)
    # no fish bucket in this container -- keep artifacts local
    from concourse import bass_utils as bu

    bu.upload_artifacts = lambda tmpdir: tmpdir


def run(inputs: dict, trace: bool = False, tmpdir: str | None = None):
    """Returns (out [8, 2048, 64] f32, exec_time_ns or None)."""
    x = np.asarray(inputs["x"], dtype=np.float32)
    wqkv = np.ascontiguousarray(
        np.stack([np.asarray(inputs["Wq"]), np.asarray(inputs["Wk"]),
                  np.asarray(inputs["Wv"])]).astype(np.float32)
    )
    nc = build_nc()
    if trace:
        _install_profile_hook()
    in_maps = [
        {"xT": np.ascontiguousarray(x[b].T), "wqkv": wqkv} for b in range(B)
    ]
    res = run_bass_kernel_spmd(
        nc, in_maps, core_ids=list(range(NCORES)), trace=trace, tmpdir=tmpdir
    )
    out = np.stack([res.results[b]["out"] for b in range(B)]).astype(np.float32)
    return out, res.exec_time_ns


def kernel(**inputs) -> np.ndarray:
    out, _ = run(inputs)
    return out


if __name__ == "__main__":
    rng = np.random.default_rng(0)
    ins = {
        "x": rng.standard_normal((B, T, E), dtype=np.float32),
        "Wq": rng.uniform(-1 / 32, 1 / 32, (E, H)).astype(np.float32),
        "Wk": rng.uniform(-1 / 32, 1 / 32, (E, H)).astype(np.float32),
        "Wv": rng.uniform(-1 / 32, 1 / 32, (E, H)).astype(np.float32),
    }
    o, ns = run(ins, trace=False)
    print("out", o.shape, o.dtype, "exec_ns", ns)


# revision 11
# speedup vs baseline: 1.0252x; 1.0252x over previous
"""Trainium2 Bass kernel for a single causal attention head.

Problem: x [8, 2048, 1024] f32, Wq/Wk/Wv [1024, 64] f32 ->
         out [8, 2048, 64] f32  (causal softmax(q k^T / 8) v per batch)

Strategy: data-parallel over batch -- core b computes batch element b,
no collectives. Per core, a column-pipelined flash-style kernel:
the T axis is split in 4 slices of 512; loading x^T slice j unlocks
the projections, score column j and PV windows of column j, so DMA
overlaps compute across columns. PV of column j is emitted after the
projections+scores of column j+1 so the TensorE stream has work while
ScalarE drains the exp queue.

Layouts (bf16 compute, f32 PSUM accumulation):
  x_pre    [4, 128, 8, 512] f32 in DRAM (host-marshalled x^T slices,
           fully contiguous per partition), cast to bf16 on load (SWDGE)
  w_pre    [128, 8, 3, 64] f32 (host-marshalled), HWDGE load + DVE cast
  qT | kT  [64, T] each, computed as one M=128 matmul with lhsT=[Wq|Wk]
  scoresT  [s-chunk 128, t 512] matmul (K=h=64)
  exp      bf16 tiles; causal staircase via block skipping + one
           [128,128] lower-triangle multiplicative mask on the diagonal
  PV       out[t 128, 65] = expT_block^T @ [v | 1]; column 64 gives the
           softmax denominator for free; normalize via reciprocal +
           per-partition tensor_scalar multiply.
  out_pre  [4, 128, 4, 64] f32 in DRAM, host reassembles to [T, H].
"""

import sys
from contextlib import ExitStack

sys.path.insert(0, "/opt/trn_rl_repo")

import numpy as np

import concourse.bass as bass
import concourse.tile as tile
from concourse import bacc, mybir
from concourse.bass_utils import run_bass_kernel_spmd

B, T, E, H = 8, 2048, 1024, 64
NCORES = 8
TJ = 512            # t-slice width (score tile free dim)
NJ = T // TJ        # 4 columns
BF16 = mybir.dt.bfloat16
F32 = mybir.dt.float32


def build_kernel(tc: "tile.TileContext", out: bass.AP, xp_dram: bass.AP,
                 wp_dram: bass.AP):
    nc = tc.nc
    EXP = mybir.ActivationFunctionType.Exp
    MUL = mybir.AluOpType.mult

    ctx = ExitStack()
    const = ctx.enter_context(tc.tile_pool(name="const", bufs=1))
    xp = ctx.enter_context(tc.tile_pool(name="xp", bufs=3))
    expp = ctx.enter_context(tc.tile_pool(name="expp", bufs=30))
    outp = ctx.enter_context(tc.tile_pool(name="outp", bufs=2))
    small = ctx.enter_context(tc.tile_pool(name="small", bufs=4))
    ps_qk = ctx.enter_context(tc.tile_pool(name="ps_qk", bufs=2, space="PSUM"))
    ps_v = ctx.enter_context(tc.tile_pool(name="ps_v", bufs=2, space="PSUM"))
    ps_s = ctx.enter_context(tc.tile_pool(name="ps_s", bufs=2, space="PSUM"))
    ps_o = ctx.enter_context(tc.tile_pool(name="ps_o", bufs=2, space="PSUM"))

    # Weights: HWDGE f32 load (keeps the SWDGE queue free for x), DVE cast.
    # Layout [128, ec, r, h]: [Wq|Wk] of one e-chunk is a contiguous
    # 128-wide free slice (lhsT of the combined qk projection).
    w_f32 = const.tile([128, 8, 3, H], F32, tag="wf")
    nc.sync.dma_start(w_f32[:], wp_dram[:])
    w_sb = const.tile([128, 8, 3, H], BF16, tag="w")
    nc.vector.tensor_copy(w_sb[:], w_f32[:])

    # Lower-triangular multiplicative mask: tri[p, f] = 1 if p <= f else 0.
    # (built inverted: fill=0.0 would hit the uninitialized Pool_zero reg)
    tri = const.tile([128, 128], BF16, tag="tri")
    nc.gpsimd.memset(tri[:], 0.0)
    nc.gpsimd.affine_select(
        out=tri[:], in_=tri[:],
        compare_op=mybir.AluOpType.is_ge, fill=1.0,
        base=-1, pattern=[[-1, 128]], channel_multiplier=1,
    )

    qk_tiles = []   # per column: [128, TJ] bf16, rows 0:64 = qT, 64:128 = kT
    kt_tiles = []   # per column: [64, TJ] bf16 (kT shifted to partitions 0-63)
    v_tiles = []    # per column: [128, 4, H+1] bf16 ([v | ones])
    for j in range(NJ):
        qk_tiles.append(const.tile([128, TJ], BF16, tag=f"qk{j}", name=f"qk{j}"))
        kt_tiles.append(const.tile([64, TJ], BF16, tag=f"kt{j}", name=f"kt{j}"))
        v_tiles.append(const.tile([128, 4, H + 1], BF16, tag=f"v{j}", name=f"v{j}"))

    etiles = {}     # (j, i) -> exp tile

    def emit_column(j):
        """x load, projections, scores+exp for column j."""
        qk_j, kt_j, v_j = qk_tiles[j], kt_tiles[j], v_tiles[j]

        # load x^T slice j (all 8 e-chunks of t-range jsl), cast bf16
        xsl = xp.tile([128, 8, TJ], BF16, tag="x", name=f"x{j}")
        nc.gpsimd.dma_start(xsl[:], xp_dram[j])

        # combined q/k projection: psum[0:64]=qT, [64:128]=kT
        psq = ps_qk.tile([128, TJ], F32, tag="qk", name=f"psq{j}")
        for ec in range(8):
            nc.tensor.matmul(
                psq[:], w_sb[:, ec, 0:2, :], xsl[:, ec, :],
                start=(ec == 0), stop=(ec == 7),
            )
        nc.vector.tensor_copy(qk_j[:], psq[:])
        # move kT rows down to partitions 0-63 (matmul lhsT needs base 0)
        nc.sync.dma_start(kt_j[:], qk_j[64:128, :])

        # v projection for s-chunks 4j..4j+3, plus ones column
        nc.vector.memset(v_j[:, :, H], 1.0)
        for c in range(4):
            psv = ps_v.tile([128, H], F32, tag="v", name=f"psv{j}_{c}")
            for ec in range(8):
                nc.tensor.matmul(
                    psv[:], xsl[:, ec, c * 128:(c + 1) * 128], w_sb[:, ec, 2, :],
                    start=(ec == 0), stop=(ec == 7),
                )
            nc.vector.tensor_copy(v_j[:, c, 0:H], psv[:])

        # score column j: scoresT[s-chunk i, t in jsl], exp, diagonal mask
        for i in range(4 * j + 4):
            r = i - 4 * j          # r >= 0 -> staircase block
            f0 = 128 * r if r > 0 else 0
            pss = ps_s.tile([128, TJ], F32, tag="s", name=f"pss{j}_{i}")
            nc.tensor.matmul(
                pss[:, f0:], kt_tiles[i // 4][:, (i % 4) * 128:(i % 4 + 1) * 128],
                qk_j[0:64, f0:], start=True, stop=True,
            )
            e = expp.tile([128, TJ], BF16, tag="e", name=f"e{j}_{i}")
            nc.scalar.activation(e[:, f0:], pss[:, f0:], EXP, scale=0.125)
            if r >= 0:
                # only the diagonal 128-wide window is partially valid
                nc.vector.tensor_tensor(
                    e[:, 128 * r:128 * (r + 1)], e[:, 128 * r:128 * (r + 1)],
                    tri[:], op=MUL,
                )
            etiles[(j, i)] = e

    def emit_pv(j):
        """PV + normalize + store for the 4 output windows of column j."""
        osb = outp.tile([128, 4, H], F32, tag="o", name=f"osb{j}")
        for c in range(4):
            m = 4 * j + c
            po = ps_o.tile([128, H + 1], F32, tag="po", name=f"po{j}_{c}")
            for i in range(m + 1):
                nc.tensor.matmul(
                    po[:], etiles[(j, i)][:, c * 128:(c + 1) * 128],
                    v_tiles[i // 4][:, i % 4, :],
                    start=(i == 0), stop=(i == m),
                )
            rec = small.tile([128, 1], F32, tag="rec", name=f"rec{j}_{c}")
            nc.vector.reciprocal(rec[:], po[:, H:H + 1])
            nc.vector.tensor_scalar_mul(osb[:, c, :], po[:, 0:H], rec[:])
        nc.sync.dma_start(out[j], osb[:])

    # PV of column j is emitted after column j+1's scores: TensorE then has
    # projection/score work to run while ScalarE drains column j's exps.
    emit_column(0)
    for j in range(1, NJ):
        emit_column(j)
        emit_pv(j - 1)
    emit_pv(NJ - 1)

    ctx.close()


_NC_CACHE = None


def build_nc():
    global _NC_CACHE
    if _NC_CACHE is not None:
        return _NC_CACHE
    nc = bacc.Bacc(
        "TRN2", target_bir_lowering=False, debug=False,
        enable_asserts=False, num_devices=NCORES,
    )
    xp_dram = nc.dram_tensor("xp", [NJ, 128, 8, TJ], F32, kind="ExternalInput").ap()
    wp_dram = nc.dram_tensor("wp", [128, 8, 3, H], F32, kind="ExternalInput").ap()
    out = nc.dram_tensor("out", [NJ, 128, 4, H], F32, kind="ExternalOutput").ap()
    with tile.TileContext(nc) as tc:
        build_kernel(tc, out, xp_dram, wp_dram)
    nc.finalize()
    _NC_CACHE = nc
    return nc


def _marshal(x_b: np.ndarray, wqkv: np.ndarray):
    # x_pre[j, p, ec, t'] = x[j*TJ + t', ec*128 + p]
    xp_in = np.ascontiguousarray(
        x_b.reshape(NJ, TJ, 8, 128).transpose(0, 3, 2, 1)
    )
    return xp_in


def _install_profile_hook():
    """The agent image lacks ``antenv.axon_hooks``; inject a shim so
    run_bass_kernel_spmd(trace=True) can reach the axon NTFF profiler."""
    import types

    if "antenv.axon_hooks" not in sys.modules:
        mod = types.ModuleType("antenv.axon_hooks")
        holder = {}
        mod.set_axon_ntff_profile_hook = lambda h: holder.__setitem__("h", h)
        mod.get_axon_ntff_profile_hook = lambda: holder.get("h")
        sys.modules["antenv.axon_hooks"] = mod
    from trn_agent_boot.trn_boot import _ntff_profile_via_ctypes

    hook = _ntff_profile_via_ctypes("/opt/axon/libaxon_pjrt.so")
    sys.modules["antenv.axon_hooks"].set_axon_ntff_profile_hook(hook)
    # no fish bucket in this container -- keep artifacts local
    from concourse import bass_utils as bu

    bu.upload_artifacts = lambda tmpdir: tmpdir


def run(inputs: dict, trace: bool = False, tmpdir: str | None = None):
    """Returns (out [8, 2048, 64] f32, exec_time_ns or None)."""
    x = np.asarray(inputs["x"], dtype=np.float32)
    # w_pre[p, ec, r, h] = W_r[ec*128 + p, h]
    wqkv = np.stack([np.asarray(inputs["Wq"]), np.asarray(inputs["Wk"]),
                     np.asarray(inputs["Wv"])]).astype(np.float32)
    w_pre = np.ascontiguousarray(wqkv.reshape(3, 8, 128, H).transpose(2, 1, 0, 3))
    nc = build_nc()
    if trace:
        _install_profile_hook()
    in_maps = [{"xp": _marshal(x[b], wqkv), "wp": w_pre} for b in range(B)]
    res = run_bass_kernel_spmd(
        nc, in_maps, core_ids=list(range(NCORES)), trace=trace, tmpdir=tmpdir
    )
    # out_pre[j, p, c, h] -> out[t = j*512 + c*128 + p, h]
    out = np.stack([
        res.results[b]["out"].transpose(0, 2, 1, 3).reshape(T, H)
        for b in range(B)
    ]).astype(np.float32)
    return out, res.exec_time_ns


def kernel(**inputs) -> np.ndarray:
    out, _ = run(inputs)
    return out


if __name__ == "__main__":
    rng = np.random.default_rng(0)
    ins = {
        "x": rng.standard_normal((B, T, E), dtype=np.float32),
        "Wq": rng.uniform(-1 / 32, 1 / 32, (E, H)).astype(np.float32),
        "Wk": rng.uniform(-1 / 32, 1 / 32, (E, H)).astype(np.float32),
        "Wv": rng.uniform(-1 / 32, 1 / 32, (E, H)).astype(np.float32),
    }
    o, ns = run(ins, trace=False)
    print("out", o.shape, o.dtype, "exec_ns", ns)
